# revision 1
# baseline (speedup 1.0000x reference)
"""HINormer sparse-attention kernel for Trainium2 (8 NeuronCores, SPMD).

Math (reference reformulated):
  softmax_t(sl[s] + sr[t] + bil[s,t]) == softmax_t(sr[t] + bil[s,t])
    -> the whole fl = h@Wl / al branch cancels (constant per softmax row).

Sharding: core c -> (batch b = c//2, query-half q = c%2). Each core computes
complete output rows LN(h + fh) for its 1024 query rows; no collectives.

Per-core dataflow (all matmuls bf16, PSUM fp32):
  S1: fr[t, d'] = hT.T @ Wr; evac -> frO (bf16, ones col per head at 64);
      leaky/ar-mult on GPSIMD, reduce on DVE -> sr_all[t-tile, head].
  S2: rq[hd_pair, t] = Wrt_pair.T @ rhT ; rk[hd_pair, s_q] = Wrs_pair.T @ rhTq
  S3 per head: psb[t,s] = rq_h.T @ rk_h (K=64); pt = exp(psb + sr bias) [ACT];
      flipped ctx: psc[s, 0:65] += pt[t-tile, s-blk].T @ frO_h[t-tile, 0:65]
      (col 64 = softmax denominator via the ones column, N=65 -> cheap).
  Normalize on evac (DVE tensor_scalar by 1/den); transpose to hsaT[d', s]
      via DMA-XBAR (pairs 0-2, off the critical path) or PE identity-matmul
      + ACT evac (pair 3, tail latency); S4: fh = hsaT.T @ Wf; LN with mean
      from an N=1 matmul column (hsaT @ rowsum(Wf) + host hsum), variance
      via ACT Square+accum, (g==1, b==0 fast path) + out.

Engine budget (TimelineSim): ACT ~138us busy (exp is the bottleneck:
128 x [128,1024] tiles at ~1038ns each), PE ~110us, DVE ~70us, GPSIMD
~19us; modeled per-core time 167.7us vs 218.2us baseline.
"""

import sys

for _p in ("/opt/trn_rl_repo",):
    if _p not in sys.path:
        sys.path.append(_p)

import numpy as np
import ml_dtypes

BF16 = ml_dtypes.bfloat16

B, S, D = 4, 2048, 512
H, HD, RL = 8, 64, 64
SLOPE = 0.01
LN_EPS = 1e-5
NCORES = 8
SQ = S // 2          # 1024 query rows per core
KT = S // 128        # 16 key/t tiles
MQ = SQ // 128       # 8 query s-blocks
DK = D // 128        # 4 d-tiles
NP = H // 2          # 4 head pairs

_CACHE = {}


def _build(apply_gb):
    import concourse.bacc as bacc
    import concourse.tile as tile
    import concourse.bass as bass
    from concourse import mybir

    f32 = mybir.dt.float32
    bf16 = mybir.dt.bfloat16
    Exp = mybir.ActivationFunctionType.Exp
    Sqrt = mybir.ActivationFunctionType.Sqrt
    Square = mybir.ActivationFunctionType.Square
    Alu = mybir.AluOpType
    AxX = mybir.AxisListType.X

    nc = bacc.Bacc("TRN2", target_bir_lowering=False, debug=False,
                   num_devices=NCORES)

    def din(name, shape, dt):
        return nc.dram_tensor(name, shape, dt, kind="ExternalInput").ap()

    hT = din("hT", [D, S], bf16)          # h[b].T
    hrows = din("hrows", [SQ, D], f32)    # h[b, s_rows] (residual, fp32)
    rhT = din("rhT", [RL, S], bf16)       # rh[b].T
    rhTq = din("rhTq", [RL, SQ], bf16)    # rh[b, s_rows].T
    Wr_d = din("Wr", [D, D], bf16)
    Wrs_d = din("Wrs", [RL, D], bf16)     # cols already head-major
    Wrt_d = din("Wrt", [RL, D], bf16)
    Wf_d = din("Wf", [D, D], bf16)
    arv = din("arv", [D], f32)            # ar tiled per head
    hsum_d = din("hsum", [SQ], f32)       # sum_d h[b, s, :] (residual rowsum)
    wf1_d = din("wf1", [D], bf16)         # sum_d' Wf[d, d'] (rowsum)
    ident_d = din("ident", [128, 128], bf16)  # for PE-transpose at the tail
    if apply_gb:
        g_d = din("g", [D], f32)
        b_d = din("b", [D], f32)
    out = nc.dram_tensor("out", [SQ, D], f32, kind="ExternalOutput").ap()

    def bcast_ap(src_ap, parts, free):
        return bass.AP(tensor=src_ap.tensor, offset=src_ap.offset,
                       ap=[[0, parts], [1, free]])

    with tile.TileContext(nc) as tc:
        with tc.tile_pool(name="singles", bufs=1) as singles:
            # ---- inputs on the S1 critical path first (sr0 gates ACT) ----
            Wr_sb = singles.tile([128, DK, D], bf16)
            nc.sync.dma_start(out=Wr_sb,
                              in_=Wr_d.rearrange("(k p) n -> p k n", p=128))
            hTs = singles.tile([128, DK, S], bf16)
            hT_sb = [hTs[:, k, :] for k in range(DK)]
            hT_pkt = hT.rearrange("(k p) t -> p k t", p=128)
            # chunked loads, first t-chunk of every k first (S1 i=0 dep)
            nc.sync.dma_start(out=hTs[:, :, 0:512], in_=hT_pkt[:, :, 0:512])
            # S2-pair-0 inputs next
            rhT_sb = singles.tile([RL, S], bf16)
            nc.sync.dma_start(out=rhT_sb, in_=rhT)
            rhTq_sb = singles.tile([RL, SQ], bf16)
            nc.sync.dma_start(out=rhTq_sb, in_=rhTq)
            Wrt_sb = singles.tile([RL, D], bf16)
            nc.sync.dma_start(out=Wrt_sb, in_=Wrt_d)
            Wrs_sb = singles.tile([RL, D], bf16)
            nc.sync.dma_start(out=Wrs_sb, in_=Wrs_d)
            ar_bc = singles.tile([128, D], f32)
            nc.gpsimd.dma_start(out=ar_bc, in_=bcast_ap(arv, 128, D))
            for c in range(1, 4):
                nc.sync.dma_start(out=hTs[:, :, 512 * c:512 * (c + 1)],
                                  in_=hT_pkt[:, :, 512 * c:512 * (c + 1)])
            Wf_sb = singles.tile([128, DK, D], bf16)
            nc.sync.dma_start(out=Wf_sb,
                              in_=Wf_d.rearrange("(k p) n -> p k n", p=128))
            hrows_v = hrows.rearrange("(m p) d -> m p d", p=128)
            hr_sb = singles.tile([128, MQ, D], f32)
            for mi in range(MQ):
                nc.sync.dma_start(out=hr_sb[:, mi, :], in_=hrows_v[mi])
            hsum_sb = singles.tile([128, MQ], f32)
            nc.sync.dma_start(out=hsum_sb,
                              in_=hsum_d.rearrange("(m p) -> p m", p=128))
            wf1_sb = singles.tile([128, DK], bf16)
            nc.sync.dma_start(out=wf1_sb,
                              in_=wf1_d.rearrange("(k p) -> p k", p=128))
            ident_sb = singles.tile([128, 128], bf16)
            nc.sync.dma_start(out=ident_sb, in_=ident_d)
            if apply_gb:
                g_bc = singles.tile([128, D], f32)
                nc.gpsimd.dma_start(out=g_bc, in_=bcast_ap(g_d, 128, D))
                b_bc = singles.tile([128, D], f32)
                nc.gpsimd.dma_start(out=b_bc, in_=bcast_ap(b_d, 128, D))
            eps_t = singles.tile([128, 1], f32)
            nc.vector.memset(eps_t, LN_EPS)
            # hoist the (single) ACT table load to t=0
            actwarm = singles.tile([128, 1], f32)
            nc.scalar.activation(out=actwarm, in_=eps_t, func=Exp)
            # PE p-state warmup: a dense dummy matmul stream from t~0 so the
            # ramp model reaches full clock before the real S1/S2 matmuls
            pewarm = singles.tile([128, 128], bf16)
            nc.vector.memset(pewarm, 0.0)

            # frO: [t-part, ti, head, (64 fr cols + ones col)] bf16
            frO = singles.tile([128, KT, H, HD + 1], bf16)
            for h in range(H):
                nc.vector.memset(frO[:, :, h, HD:HD + 1], 1.0)
            sr_all = singles.tile([128, KT, H], f32)

            rq_sb, rk_sb, hsaT = [], [], []
            for j in range(NP):
                t = singles.tile([128, S], bf16, name=f"rq{j}")
                rq_sb.append(t)
                t = singles.tile([128, SQ], bf16, name=f"rk{j}")
                rk_sb.append(t)
                t = singles.tile([128, SQ], bf16, name=f"hsaT{j}")
                hsaT.append(t)

            # ---- long-lived work pools ----
            psb_cm = tc.tile_pool(name="psb", bufs=3, space="PSUM")
            psbp = psb_cm.__enter__()
            pt_cm = tc.tile_pool(name="ptp", bufs=3)
            ptp = pt_cm.__enter__()
            sb_cm = tc.tile_pool(name="sbp", bufs=2)
            sbp = sb_cm.__enter__()
            hsap_cm = tc.tile_pool(name="hsapp", bufs=2)
            hsapp = hsap_cm.__enter__()
            recd_cm = tc.tile_pool(name="recdp", bufs=8)
            recdp = recd_cm.__enter__()

            pt_tiles = {}
            hsap_tiles = {}

            def bil_mms(h, ti, psbt):
                j, off = h // 2, 64 * (h % 2)
                for c in range(2):
                    nc.tensor.matmul(
                        psbt[:, 512 * c:512 * (c + 1)],
                        lhsT=rq_sb[j][off:off + 64, 128 * ti:128 * (ti + 1)],
                        rhs=rk_sb[j][off:off + 64, 512 * c:512 * (c + 1)],
                        start=True, stop=True)

            def exp_tile(h, ti, psbt):
                nc.scalar.activation(out=pt_tiles[h][:, ti, :], in_=psbt,
                                     func=Exp, bias=sr_all[:, ti, h:h + 1])

            CTX_ORDER = [0, 4, 1, 5, 2, 6, 3, 7]  # alternate psc banks

            def ctx_chain(h, sblk, psc_t):
                j, off = h // 2, 64 * (h % 2)
                sb4 = sblk % 4
                pt_t = pt_tiles[h]
                for tj in range(KT):
                    nc.tensor.matmul(
                        psc_t[:, sb4, 0:HD + 1],
                        lhsT=pt_t[:, tj, 128 * sblk:128 * (sblk + 1)],
                        rhs=frO[:, tj, h, :],
                        start=(tj == 0), stop=(tj == KT - 1))
                recd = recdp.tile([128, 1], f32, tag="recd", name="recd")
                nc.vector.reciprocal(recd, psc_t[:, sb4, HD:HD + 1])
                nc.vector.tensor_scalar(
                    out=hsap_tiles[j][:, sblk, off:off + 64],
                    in0=psc_t[:, sb4, 0:HD],
                    scalar1=recd, scalar2=None,
                    op0=Alu.mult)

            def pair_transposes(j):
                for sblk in range(MQ):
                    nc.sync.dma_start_transpose(
                        out=hsaT[j][:, 128 * sblk:128 * (sblk + 1)],
                        in_=hsap_tiles[j][:, sblk, :])

            def s2_unit(j, u, ps_pool, tag, evac="dve"):
                # one (matmul, evac) unit of S2 pair j; u in 0..5
                ps = ps_pool.tile([128, 512], f32, tag=tag, name="ps")
                if u < 4:
                    nc.tensor.matmul(ps, lhsT=Wrt_sb[:, 128 * j:128 * (j + 1)],
                                     rhs=rhT_sb[:, 512 * u:512 * (u + 1)],
                                     start=True, stop=True)
                    dst = rq_sb[j][:, 512 * u:512 * (u + 1)]
                else:
                    n = u - 4
                    nc.tensor.matmul(ps, lhsT=Wrs_sb[:, 128 * j:128 * (j + 1)],
                                     rhs=rhTq_sb[:, 512 * n:512 * (n + 1)],
                                     start=True, stop=True)
                    dst = rk_sb[j][:, 512 * n:512 * (n + 1)]
                if evac == "act":
                    nc.scalar.copy(out=dst, in_=ps)
                else:
                    nc.vector.tensor_copy(out=dst, in_=ps)

            def emit_s2_pair(j, ps_pool, tag="ps12"):
                for u in range(6):
                    s2_unit(j, u, ps_pool, tag)

            # ============ Phase A/B: S2, S1 + bil/exp of heads 0,1 ===========
            with tc.tile_pool(name="ps12", bufs=2, space="PSUM") as ps12:
                for w in range(24):
                    pw = ps12.tile([128, 512], f32, tag="ps12", name="pw")
                    nc.tensor.matmul(pw[:, 0:128], lhsT=pewarm, rhs=pewarm,
                                     start=True, stop=True)
                pt_tiles[0] = ptp.tile([128, KT, SQ], bf16, tag="pt", name="pt0")
                pt_tiles[1] = ptp.tile([128, KT, SQ], bf16, tag="pt", name="pt1")

                def s1_mm_copy(i):
                    ps = ps12.tile([128, 512], f32, tag="ps12", name="ps")
                    for k in range(DK):
                        nc.tensor.matmul(ps,
                                         lhsT=hT_sb[k][:, 128 * i:128 * (i + 1)],
                                         rhs=Wr_sb[:, k, :],
                                         start=(k == 0), stop=(k == DK - 1))
                    nc.vector.tensor_copy(
                        out=frO[:, i, :, 0:HD],
                        in_=ps.rearrange("p (h c) -> p h c", c=HD))

                def s1_leaky_mult(i):
                    lk = sbp.tile([128, H, HD], bf16, tag="lk", name="lk")
                    fr_i = frO[:, i, :, 0:HD]
                    nc.vector.scalar_tensor_tensor(
                        out=lk, in0=fr_i, scalar=SLOPE, in1=fr_i,
                        op0=Alu.mult, op1=Alu.max)
                    lka = sbp.tile([128, H, HD], f32, tag="lka", name="lka")
                    # first tiles: keep the whole chain on DVE to skip the
                    # Q7 launch + cross-engine latency (sr0 gates the train)
                    eng = nc.vector if i < 2 else nc.gpsimd
                    eng.tensor_mul(
                        lka, lk, ar_bc.rearrange("p (h c) -> p h c", c=HD))
                    return lka

                def sr_finish(i, lka):
                    nc.vector.reduce_sum(out=sr_all[:, i, :], in_=lka, axis=AxX)

                # two-stage software pipeline: the sr chain (copy -> leaky ->
                # gpsimd mult -> reduce) has ~3.4us latency, so bil/exp and
                # the sr reduce for tile i trail the S1 step by 2
                lkas = {}
                for i in range(KT):
                    s1_mm_copy(i)
                    if i == 0:
                        # pair-0 after S1-i0 (S1 feeds sr0, the ACT gate);
                        # all its evacs on the still-idle ACT engine
                        for u in (0, 4, 5):
                            s2_unit(0, u, ps12, "ps12", evac="act")
                    elif i == 1:
                        for u in (1, 2, 3):
                            s2_unit(0, u, ps12, "ps12", evac="act")
                    if i - 2 in lkas:
                        sr_finish(i - 2, lkas.pop(i - 2))
                    lkas[i] = s1_leaky_mult(i)
                    if i in (6, 8, 10):
                        s2_unit(1, (0, 4, 5)[(i - 6) // 2], ps12, "ps12")
                    if i >= 2:
                        for h in (0, 1):
                            psbt = psbp.tile([128, SQ], f32, tag="psb",
                                             name="psb")
                            bil_mms(h, i - 2, psbt)
                            exp_tile(h, i - 2, psbt)
                for i in (KT - 2, KT - 1):
                    sr_finish(i, lkas.pop(i))
                    for h in (0, 1):
                        psbt = psbp.tile([128, SQ], f32, tag="psb", name="psb")
                        bil_mms(h, i, psbt)
                        exp_tile(h, i, psbt)

            # ============ Phase C: heads 2..7 with trailing ctx ============
            psc_cm = tc.tile_pool(name="pscp", bufs=2, space="PSUM")
            pscp = psc_cm.__enter__()
            psc_tiles = {}

            def start_ctx_part(h, sblk):
                j = h // 2
                if sblk == 0 and h % 2 == 0:
                    hsap_tiles[j] = hsapp.tile([128, MQ, 128], bf16,
                                               tag="hsap", name=f"hsap{j}")
                if sblk % 4 == 0:
                    psc_tiles[(h, sblk // 4)] = pscp.tile(
                        [128, 4, 128], f32, tag="psc", name=f"psc{h}_{sblk}")

            for h in range(2, H):
                pt_tiles[h] = ptp.tile([128, KT, SQ], bf16, tag="pt",
                                       name=f"pt{h}")
                # ctx work interleaved into this head's bil/exp window
                pending = [(h - 1, s) for s in CTX_ORDER]
                if h == 2:
                    pending = [(0, s) for s in CTX_ORDER] + pending
                per_ti = [[] for _ in range(KT)]
                for idx, work in enumerate(pending):
                    per_ti[(idx * KT) // len(pending)].append(work)
                for ti in range(KT):
                    psbt = psbp.tile([128, SQ], f32, tag="psb", name="psb")
                    bil_mms(h, ti, psbt)
                    exp_tile(h, ti, psbt)
                    # spread S2 for upcoming pairs through the windows
                    if h == 2 and ti in (1, 3, 5):
                        s2_unit(1, (1, 2, 3)[ti // 2], pscp, "psc")
                    if h == 3 and ti % 2 == 1 and ti < 12:
                        s2_unit(2, ti // 2, pscp, "psc")
                    if h == 5 and ti % 2 == 1 and ti < 12:
                        s2_unit(3, ti // 2, pscp, "psc")
                    for (ch, cs) in per_ti[ti]:
                        start_ctx_part(ch, cs)
                        ctx_chain(ch, cs, psc_tiles[(ch, cs // 4)])
                        if cs == CTX_ORDER[-1] and ch % 2 == 1:
                            pair_transposes(ch // 2)

            # trailing ctx for head 7
            for sblk in CTX_ORDER:
                start_ctx_part(H - 1, sblk)
                ctx_chain(H - 1, sblk, psc_tiles[(H - 1, sblk // 4)])

            psc_cm.__exit__(None, None, None)
            recd_cm.__exit__(None, None, None)
            hsap_cm.__exit__(None, None, None)
            sb_cm.__exit__(None, None, None)
            pt_cm.__exit__(None, None, None)
            psb_cm.__exit__(None, None, None)

            # ================= S4: fh + LN =================
            # LN stats split across engines: mean comes nearly free from an
            # N=1 matmul column (sum_d fh = hsaT @ rowsum(Wf)) plus the host
            # hsum; ACT (idle at the tail) does sum(x^2) via Square accum.
            out_v = out.rearrange("(m p) d -> m p d", p=128)
            rD = 1.0 / D
            with tc.tile_pool(name="ps_fh", bufs=2, space="PSUM") as ps_fh, \
                 tc.tile_pool(name="lnp", bufs=4) as lnp:
                # pair-3 transpose via PE identity-matmul + ACT evac: far
                # lower latency than the DMA XBAR path, and PE/ACT are
                # otherwise idle at this point
                for half in range(2):
                    mmt = ps_fh.tile([128, 4, 128], f32, tag="ptr",
                                     name=f"mmt{half}")
                    for sb4 in range(4):
                        sblk = half * 4 + sb4
                        nc.tensor.matmul(mmt[:, sb4, :],
                                         lhsT=hsap_tiles[NP - 1][:, sblk, :],
                                         rhs=ident_sb, start=True, stop=True)
                        nc.scalar.copy(
                            out=hsaT[NP - 1][:, 128 * sblk:128 * (sblk + 1)],
                            in_=mmt[:, sb4, :])
                sfh = ps_fh.tile([128, MQ], f32, tag="sfh", name="sfh",
                                 bufs=1)
                for mi in range(MQ):
                    psf = ps_fh.tile([128, 512], f32, tag="fh", name="fh")
                    for j in range(NP):
                        nc.tensor.matmul(psf,
                                         lhsT=hsaT[j][:, 128 * mi:128 * (mi + 1)],
                                         rhs=Wf_sb[:, j, :],
                                         start=(j == 0), stop=(j == NP - 1))
                    for j in range(NP):
                        nc.tensor.matmul(sfh[:, mi:mi + 1],
                                         lhsT=hsaT[j][:, 128 * mi:128 * (mi + 1)],
                                         rhs=wf1_sb[:, j:j + 1],
                                         start=(j == 0), stop=(j == NP - 1))
                    # mu = (hsum + sum_d fh) / D  (before xs so the ACT
                    # Square round-trip overlaps the var prep)
                    mu = lnp.tile([128, 1], f32, tag="mu", name="mu")
                    nc.vector.tensor_scalar(out=mu, in0=sfh[:, mi:mi + 1],
                                            scalar1=hsum_sb[:, mi:mi + 1],
                                            scalar2=rD,
                                            op0=Alu.add, op1=Alu.mult)
                    xs = lnp.tile([128, D], f32, tag="xs", name="xs")
                    nc.vector.tensor_add(xs, psf, hr_sb[:, mi, :])
                    scr = lnp.tile([128, D], f32, tag="scr", name="scr")
                    sx2 = lnp.tile([128, 1], f32, tag="sx2", name="sx2")
                    nc.scalar.activation(out=scr, in_=xs, func=Square,
                                         accum_out=sx2)
                    # var = sx2/D - mu^2
                    nmu2 = lnp.tile([128, 1], f32, tag="nmu2", name="nmu2")
                    nc.vector.scalar_tensor_tensor(
                        out=nmu2, in0=mu, scalar=-1.0, in1=mu,
                        op0=Alu.mult, op1=Alu.mult)
                    var = lnp.tile([128, 1], f32, tag="var", name="var")
                    nc.vector.tensor_scalar(out=var, in0=sx2, scalar1=rD,
                                            scalar2=nmu2[:, 0:1],
                                            op0=Alu.mult, op1=Alu.add)
                    std = lnp.tile([128, 1], f32, tag="std", name="std")
                    nc.scalar.activation(out=std, in_=var, func=Sqrt,
                                         bias=eps_t)
                    rstd = lnp.tile([128, 1], f32, tag="rstd", name="rstd")
                    nc.vector.reciprocal(rstd, std)
                    xo = lnp.tile([128, D], f32, tag="xo", name="xo")
                    nc.vector.tensor_scalar(out=xo, in0=xs,
                                            scalar1=mu[:, 0:1], scalar2=rstd,
                                            op0=Alu.subtract, op1=Alu.mult)
                    if apply_gb:
                        nc.vector.tensor_mul(xo, xo, g_bc)
                        nc.vector.tensor_add(xo, xo, b_bc)
                    nc.sync.dma_start(out=out_v[mi], in_=xo)

    nc.compile()
    return nc


def _get_nc(apply_gb=False):
    key = ("nc", apply_gb)
    if key not in _CACHE:
        _CACHE[key] = _build(apply_gb)
    return _CACHE[key]


def _in_maps(h, rh, Wr, ar, Wrs, Wrt, Wf, ln_g, ln_b):
    h = np.asarray(h, np.float32)
    rh = np.asarray(rh, np.float32)
    apply_gb = not (np.all(np.asarray(ln_g) == 1.0)
                    and np.all(np.asarray(ln_b) == 0.0))
    in_maps = []
    for c in range(NCORES):
        b, q = c // 2, c % 2
        sl = slice(q * SQ, (q + 1) * SQ)
        m = {
            "hT": np.ascontiguousarray(h[b].T).astype(BF16),
            "hrows": np.ascontiguousarray(h[b, sl]),
            "rhT": np.ascontiguousarray(rh[b].T).astype(BF16),
            "rhTq": np.ascontiguousarray(rh[b, sl].T).astype(BF16),
            "Wr": np.asarray(Wr, np.float32).astype(BF16),
            "Wrs": np.asarray(Wrs, np.float32).astype(BF16),
            "Wrt": np.asarray(Wrt, np.float32).astype(BF16),
            "Wf": np.asarray(Wf, np.float32).astype(BF16),
            "arv": np.ascontiguousarray(np.tile(np.asarray(ar, np.float32), H)),
            "hsum": np.ascontiguousarray(h[b, sl].sum(-1, dtype=np.float64)
                                         .astype(np.float32)),
            "wf1": np.ascontiguousarray(
                np.asarray(Wf, np.float32).astype(BF16).astype(np.float32)
                .sum(-1).astype(BF16)),
            "ident": np.eye(128, dtype=np.float32).astype(BF16),
        }
        if apply_gb:
            m["g"] = np.asarray(ln_g, np.float32)
            m["b"] = np.asarray(ln_b, np.float32)
        in_maps.append(m)
    return in_maps, apply_gb


def _assemble(results):
    outp = np.empty((B, S, D), np.float32)
    for c in range(NCORES):
        b, q = c // 2, c % 2
        outp[b, q * SQ:(q + 1) * SQ] = results[c]["out"]
    return outp


def kernel(h, rh, Wl, Wr, al, ar, Wrs, Wrt, Wf, ln_g, ln_b, **_ignored):
    from concourse.bass_utils import run_bass_kernel_spmd

    in_maps, apply_gb = _in_maps(h, rh, Wr, ar, Wrs, Wrt, Wf, ln_g, ln_b)
    nc = _get_nc(apply_gb)
    res = run_bass_kernel_spmd(nc, in_maps, core_ids=list(range(NCORES)))
    _CACHE["last_results"] = res
    return _assemble(res.results)

